# revision 5
# baseline (speedup 1.0000x reference)
"""MoE classifier kernel for Trainium2 (8 NeuronCores, Bass/Tile).

Strategy (expert-major sparse routing):
  - Host: gate logits in fp64 -> top-3 -> sparse softmax weights gw.
  - For each expert e, gather the tokens routed to it (~B*K/E each),
    split across 8 cores uniformly (same per-core token count T[e] on
    every core -> single SPMD program), pad with zero tokens.
  - Device (per core): for each expert segment, dense 2-layer FFN on the
    gathered tokens in bf16 (fp32 PSUM accumulate):
        hT = gelu(W1[e] @ xT + b1[e])        [H, T] (exact erf-gelu LUT)
        out = (hT.T @ W2[e].T + b2[e]) * gw  [T, C] (gw fused into the
                                              PSUM->SBUF copy as a
                                              per-partition scale)
  - Host: scatter-add the 3 weighted expert partials per token.

All expert-FFN FLOPs (99.9% of total) run on device; only the tiny gate
matmul (0.07%) and the final scatter-add (0.008%) are host-side glue.
"""

import numpy as np
import ml_dtypes

B, D, H, E, C, TOPK = 16384, 2048, 2048, 6, 1024, 3
N_CORES = 8
BF16 = ml_dtypes.bfloat16

_CACHE = {}


def _route(x, gate_W, gate_b):
    """fp64 gating. Returns gw [B, E] fp32 and per-expert token lists."""
    logits = x.astype(np.float64) @ gate_W.astype(np.float64).T + gate_b.astype(
        np.float64
    )
    order = np.argsort(-logits, axis=1)
    top3 = order[:, :TOPK]  # [B, 3]
    rows = np.arange(B)[:, None]
    vals = logits[rows, top3]  # [B, 3]
    vals = vals - vals.max(axis=1, keepdims=True)
    ex = np.exp(vals)
    w3 = ex / ex.sum(axis=1, keepdims=True)  # [B, 3] fp64
    gw = np.zeros((B, E), np.float32)
    gw[rows, top3] = w3.astype(np.float32)
    experts_tokens = [np.nonzero(gw[:, e] > 0)[0] for e in range(E)]
    return gw, experts_tokens


def _build_program(T):
    """Build the SPMD Bass program for per-expert per-core token counts T."""
    import concourse.bacc as bacc
    import concourse.mybir as mybir
    import concourse.tile as tile

    F32 = mybir.dt.float32
    BF = mybir.dt.bfloat16
    GELU = mybir.ActivationFunctionType.Gelu
    COPY = mybir.ActivationFunctionType.Copy

    KD = D // 128  # contraction chunks for stage A
    KH = H // 128  # contraction chunks for stage B
    HG = H // 512  # 512-wide h groups of W1 tiles
    NC_ = C // 512  # 512-wide c chunks
    Ttot = sum(T)
    nblk = Ttot // 128

    nc = bacc.Bacc("TRN2", target_bir_lowering=False, debug=False)
    xt = nc.declare_dram_parameter("xt", [D, Ttot], BF, isOutput=False)
    w1t = nc.declare_dram_parameter("w1t", [E, D, H], BF, isOutput=False)
    w2t = nc.declare_dram_parameter("w2t", [E, H, C], BF, isOutput=False)
    b1c = nc.declare_dram_parameter("b1c", [128, E * KH], F32, isOutput=False)
    b2r = nc.declare_dram_parameter("b2r", [1, E * C], BF, isOutput=False)
    gwc = nc.declare_dram_parameter("gwc", [128, nblk], F32, isOutput=False)
    out = nc.declare_dram_parameter("out", [Ttot, C], F32, isOutput=True)

    T_max = max(T)

    with tile.TileContext(nc) as tc:
        with (
            tc.tile_pool(name="xt", bufs=24) as xt_pool,
            tc.tile_pool(name="hs", bufs=24) as hs_pool,
            tc.tile_pool(name="w1", bufs=24) as w1_pool,
            tc.tile_pool(name="w2", bufs=32) as w2_pool,
            tc.tile_pool(name="sm", bufs=1) as sm_pool,
            tc.tile_pool(name="ot", bufs=6) as out_pool,
            tc.tile_pool(name="psA", bufs=4, space="PSUM") as psA,
            tc.tile_pool(name="psB", bufs=3, space="PSUM") as psB,
        ):
            b1sb = sm_pool.tile([128, E * KH], F32, tag="b1")
            nc.sync.dma_start(out=b1sb[:], in_=b1c[:])
            gwsb = sm_pool.tile([128, nblk], F32, tag="gw")
            nc.sync.dma_start(out=gwsb[:], in_=gwc[:])
            b2sb = sm_pool.tile([1, E * C], BF, tag="b2")
            nc.sync.dma_start(out=b2sb[:], in_=b2r[:])
            ones1 = sm_pool.tile([1, 128], BF, tag="ones")
            nc.vector.memset(ones1[:], 1.0)

            blk0 = 0  # global block offset of current segment
            off = 0  # global token offset
            for e in range(E):
                Te = T[e]
                nb = Te // 128
                # token chunks for stage-A matmul N dim
                tch = []
                t0 = 0
                while t0 < Te:
                    tn = min(512, Te - t0)
                    tch.append((t0, tn))
                    t0 += tn

                # load gathered xT for this segment: KD tiles [128, Te]
                xts = []
                for k in range(KD):
                    t_ = xt_pool.tile([128, T_max], BF, tag="xt")
                    nc.sync.dma_start(
                        out=t_[:, :Te],
                        in_=xt[k * 128:(k + 1) * 128, off:off + Te],
                    )
                    xts.append(t_)

                # load W2t[e] resident: KH x NC_ tiles [128, 512]
                w2s = {}
                for m in range(KH):
                    for n in range(NC_):
                        t_ = w2_pool.tile([128, 512], BF, tag="w2")
                        nc.sync.dma_start(
                            out=t_[:],
                            in_=w2t[e, m * 128:(m + 1) * 128,
                                    n * 512:(n + 1) * 512],
                        )
                        w2s[(m, n)] = t_

                # Stage A: hs[m] = gelu(W1[e]_m @ xT + b1) as [128 h, Te] bf16
                hss = []
                for mg in range(HG):
                    # W1 tiles for h-group mg: KD tiles [128 d, 512 h]
                    w1s = []
                    for k in range(KD):
                        t_ = w1_pool.tile([128, 512], BF, tag="w1")
                        nc.sync.dma_start(
                            out=t_[:],
                            in_=w1t[e, k * 128:(k + 1) * 128,
                                    mg * 512:(mg + 1) * 512],
                        )
                        w1s.append(t_)
                    for ms in range(4):
                        m = mg * 4 + ms
                        hst = hs_pool.tile([128, T_max], BF, tag="hs")
                        for (t0, tn) in tch:
                            ph = psA.tile([128, 512], F32, tag="psA")
                            for k in range(KD):
                                nc.tensor.matmul(
                                    ph[:, :tn],
                                    w1s[k][:, ms * 128:(ms + 1) * 128],
                                    xts[k][:, t0:t0 + tn],
                                    start=(k == 0),
                                    stop=(k == KD - 1),
                                )
                            nc.scalar.activation(
                                hst[:, t0:t0 + tn], ph[:, :tn], GELU,
                                bias=b1sb[:, e * KH + m:e * KH + m + 1],
                            )
                        hss.append(hst)

                # Stage B: out[j] = gw[j] * (hs[:, j].T @ W2t[e] + b2[e])
                for j in range(nb):
                    jg = blk0 + j
                    for n in range(NC_):
                        po = psB.tile([128, 512], F32, tag="psB")
                        for m in range(KH):
                            nc.tensor.matmul(
                                po[:],
                                hss[m][:, j * 128:(j + 1) * 128],
                                w2s[(m, n)][:],
                                start=(m == 0),
                                stop=False,
                            )
                        nc.tensor.matmul(
                            po[:], ones1[:, :],
                            b2sb[:, e * C + n * 512:e * C + (n + 1) * 512],
                            start=False, stop=True,
                        )
                        ot = out_pool.tile([128, 512], F32, tag="ot")
                        nc.scalar.activation(
                            ot[:], po[:], COPY, scale=gwsb[:, jg:jg + 1],
                        )
                        nc.sync.dma_start(
                            out=out[jg * 128:(jg + 1) * 128,
                                    n * 512:(n + 1) * 512],
                            in_=ot[:],
                        )
                blk0 += nb
                off += Te

    nc.compile()
    return nc


def _prepare(inputs):
    """Host-side routing + per-core input maps. Returns everything needed
    to launch and to un-shard."""
    x = np.asarray(inputs["x"], np.float32)
    gate_W = np.asarray(inputs["gate_W"], np.float32)
    gate_b = np.asarray(inputs["gate_b"], np.float32)
    W1 = np.asarray(inputs["W1"], np.float32)
    b1 = np.asarray(inputs["b1"], np.float32)
    W2 = np.asarray(inputs["W2"], np.float32)
    b2 = np.asarray(inputs["b2"], np.float32)

    gw, experts_tokens = _route(x, gate_W, gate_b)

    # uniform per-core capacity per expert (multiple of 128)
    T = []
    for e in range(E):
        per_core = -(-len(experts_tokens[e]) // N_CORES)
        T.append(-(-per_core // 128) * 128)
    T = tuple(T)
    Ttot = sum(T)
    nblk = Ttot // 128
    KH = H // 128

    x_bf = x.astype(BF16)
    w1t_dev = np.ascontiguousarray(W1.transpose(0, 2, 1)).astype(BF16)  # [E,D,H]
    w2t_dev = np.ascontiguousarray(W2.transpose(0, 2, 1)).astype(BF16)  # [E,H,C]
    b1c_dev = np.ascontiguousarray(b1.reshape(E * KH, 128).T)  # [128, E*KH] f32
    b2r_dev = b2.reshape(1, E * C).astype(BF16)  # [1, E*C]

    in_maps = []
    core_tok = []  # per core: list over experts of token-id arrays
    for c in range(N_CORES):
        xt_c = np.zeros((D, Ttot), BF16)
        gw_c = np.zeros((128, nblk), np.float32)
        toks_c = []
        off = 0
        blk0 = 0
        for e in range(E):
            full = experts_tokens[e]
            n_e = len(full)
            lo = (n_e * c) // N_CORES
            hi = (n_e * (c + 1)) // N_CORES
            idx = full[lo:hi]
            toks_c.append(idx)
            n = len(idx)
            if n:
                xt_c[:, off:off + n] = x_bf[idx].T
                gwe = np.zeros(T[e], np.float32)
                gwe[:n] = gw[idx, e]
            else:
                gwe = np.zeros(T[e], np.float32)
            gw_c[:, blk0:blk0 + T[e] // 128] = gwe.reshape(T[e] // 128, 128).T
            off += T[e]
            blk0 += T[e] // 128
        core_tok.append(toks_c)
        in_maps.append({
            "xt": xt_c,
            "w1t": w1t_dev,
            "w2t": w2t_dev,
            "b1c": b1c_dev,
            "b2r": b2r_dev,
            "gwc": gw_c,
        })
    return gw, T, in_maps, core_tok


def _combine(results, T, core_tok):
    out_full = np.zeros((B, C), np.float32)
    for c in range(N_CORES):
        dev_out = results[c]["out"]
        off = 0
        for e in range(E):
            idx = core_tok[c][e]
            if len(idx):
                out_full[idx] += dev_out[off:off + len(idx)]
            off += T[e]
    return out_full


def kernel(**inputs):
    from concourse.bass_utils import run_bass_kernel_spmd

    gw, T, in_maps, core_tok = _prepare(inputs)
    if T not in _CACHE:
        _CACHE[T] = _build_program(T)
    nc = _CACHE[T]
    res = run_bass_kernel_spmd(nc, in_maps, list(range(N_CORES)))
    out_full = _combine(res.results, T, core_tok)
    return out_full, gw
